# revision 1
# baseline (speedup 1.0000x reference)
"""TRN2 Bass kernel for nn_BlockPermProduct.

The reference applies 9 probabilistic block-permutation mixing steps to each
row of x [65536, 1024]. Every step is linear in x, so the whole transform is
``out = x @ M^T`` for a 1024x1024 matrix M that depends only on the tiny
(9, 3) logits. M^T is computed on the host in float64 by pushing the identity
matrix through the reference transform; the device kernel is then a dense
row-wise matmul:

  per 128-row tile:  xT = transpose(x_tile) on the PE (8 128x128 blocks),
                     out_tile = accumulate_{c} xT_c^T @ MT_c  into PSUM,
                     copy back to SBUF, DMA out.

Matmuls and transposes run in float32r (4-byte fp32 data with tf32-class
rounding in the PE): 1 cycle/row at N=512 vs 4 for plain fp32, measured
end-to-end rel err ~2.3e-4. Loads/stores are batched as 1 MiB transfers
(two row-tiles per DMA). The PE array trace shows ~0 idle between slices;
the kernel is PE-bound at ~1.7x the per-core HBM roofline.

Sharding: pure data parallel over the batch dim across 8 cores (SPMD, no
communication); M^T is replicated.
"""

import numpy as np
from contextlib import ExitStack

import concourse.bass as bass
import concourse.bacc as bacc
import concourse.mybir as mybir
import concourse.tile as tile
from concourse.bass_utils import run_bass_kernel_spmd

BATCH = 65536
SIZE = 1024
N_CORES = 8
ROWS_PER_CORE = BATCH // N_CORES  # 8192
P = 128
PAIR = 2  # row-tiles per DMA transfer (1 MiB)
N_STEPS = ROWS_PER_CORE // (P * PAIR)  # 32
N_CHUNK = SIZE // P  # 8
HALF = 512  # PSUM bank width in fp32

F32 = mybir.dt.float32
F32R = mybir.dt.float32r

# "f32"      : plain fp32 matmuls (safest numerics, 4 cyc/row)
# "f32r"     : f32r matmuls; fp32 DMA + fp32 PE transposes, rounding to f32r
#              at the PSUM->SBUF copy
# "f32r_dma" : f32r end-to-end including DMA dtype and f32r transposes
# "xbar"     : f32r matmuls; transposes via DMA XBAR on u16 hi/lo planes
#              (PE does matmuls only)
import os as _os
MATMUL_MODE = _os.environ.get("KMODE", "f32r_dma")

TRACE = False
TRACE_KWARGS = {}
LAST_RESULTS = None

_NC_CACHE = {}


def _transform64(y, logits):
    """Float64 port of the reference transform, applied to rows of y."""
    m = 10
    sizes = [SIZE >> i for i in range(m - 1)][::-1]  # [4, 8, ..., 1024]
    out = y
    for i in range(m - 2, -1, -1):
        n = sizes[i]
        p = 1.0 / (1.0 + np.exp(-logits[i].astype(np.float64)))
        z = out.reshape(-1, n)
        sep = z.reshape(-1, n // 2, 2).transpose(0, 2, 1).reshape(-1, n)
        z = (1 - p[0]) * z + p[0] * sep
        h = n // 2
        first = (1 - p[1]) * z[:, :h] + p[1] * z[:, h - 1::-1]
        second = (1 - p[2]) * z[:, h:] + p[2] * z[:, : h - 1 : -1]
        out = np.concatenate([first, second], axis=1).reshape(out.shape)
    return out


def _build_mt(logits):
    """M^T [1024, 1024] fp32: row j = transform(e_j), so MT[j, i] = M[i, j]."""
    eye = np.eye(SIZE, dtype=np.float64)
    mt = _transform64(eye, logits)
    return np.ascontiguousarray(mt.astype(np.float32))


def _build_bass(mode):
    xbar = mode == "xbar"
    f32r_dma = mode == "f32r_dma"
    mdt = F32 if mode == "f32" else F32R
    xdt = F32R if f32r_dma else F32  # dtype of x DMA + PE transposes
    U16 = mybir.dt.uint16
    nc = bacc.Bacc("TRN2", target_bir_lowering=False, debug=False)
    x = nc.dram_tensor("x", [ROWS_PER_CORE, SIZE], xdt, kind="ExternalInput").ap()
    mt = nc.dram_tensor("mt", [SIZE, SIZE], F32 if mode == "f32r" else mdt, kind="ExternalInput").ap()
    out = nc.dram_tensor(
        "out", [ROWS_PER_CORE, SIZE], F32, kind="ExternalOutput"
    ).ap()
    identd = nc.dram_tensor("ident", [P, P], xdt, kind="ExternalInput").ap()

    with tile.TileContext(nc) as tc, ExitStack() as ctx:
        const = ctx.enter_context(tc.tile_pool(name="const", bufs=1))
        if not xbar:
            # Identity arrives from the host (f32r-typed DMA producer) so the
            # first PE transposes don't wait on gpsimd/ACT preamble chains.
            ident = const.tile([P, P], xdt, tag="ident")
            nc.sync.dma_start(ident[:], identd[:])

        xpool = ctx.enter_context(tc.tile_pool(name="xin", bufs=4))

        # Kick off the first x load BEFORE the M^T loads so the PE's first
        # transposes aren't queued behind 4 MB of constants.
        xin0 = xpool.tile([P, PAIR * SIZE], xdt, tag="xin")
        nc.sync.dma_start(
            xin0[:].rearrange("p (s n) -> p s n", n=SIZE),
            x[0 : P * PAIR, :].rearrange("(s p) n -> p s n", p=P),
        )

        # M^T resident in SBUF as 8 per-chunk tiles; each matmul depends only
        # on its own chunk's DMA, so compute overlaps the constant loads.
        mts = []
        for c in range(N_CHUNK):
            t = const.tile([P, SIZE], F32 if mode == "f32r" else mdt, tag=f"mt{c}")
            nc.sync.dma_start(t[:], mt[c * P : (c + 1) * P, :])
            if mode == "f32r":
                tr = const.tile([P, SIZE], F32R, tag=f"mtr{c}")
                nc.vector.tensor_copy(tr[:], t[:])
                t = tr
            mts.append(t)
        xtpool = ctx.enter_context(tc.tile_pool(name="xtp", bufs=4))
        opool = ctx.enter_context(tc.tile_pool(name="osb", bufs=3))
        if xbar:
            planes = ctx.enter_context(tc.tile_pool(name="planes", bufs=3))
            pso = ctx.enter_context(tc.tile_pool(name="pso", bufs=4, space="PSUM"))
        else:
            pst = ctx.enter_context(tc.tile_pool(name="pst", bufs=2, space="PSUM"))
            pso = ctx.enter_context(tc.tile_pool(name="pso", bufs=2, space="PSUM"))

        for step in range(N_STEPS):
            r0 = step * P * PAIR
            if step == 0:
                xin = xin0
            else:
                # One 1 MiB load: PAIR row-tiles side by side in the free dim.
                xin = xpool.tile([P, PAIR * SIZE], xdt, tag="xin")
                nc.sync.dma_start(
                    xin[:].rearrange("p (s n) -> p s n", n=SIZE),
                    x[r0 : r0 + P * PAIR, :].rearrange("(s p) n -> p s n", p=P),
                )
            osb = opool.tile([P, PAIR * SIZE], F32, tag="osb")

            for s in range(PAIR):
                xv = xin[:, s * SIZE : (s + 1) * SIZE]
                if xbar:
                    # Deinterleave u16 hi/lo planes (compute engines allow
                    # strided APs), transpose each plane via the DMA XBAR,
                    # re-interleave, round to f32r. PE does matmuls only.
                    xv3 = xv.bitcast(U16).rearrange("p (k two) -> p k two", two=2)
                    lo_p = planes.tile([P, SIZE], U16, tag="lop")
                    hi_p = planes.tile([P, SIZE], U16, tag="hip")
                    nc.vector.tensor_copy(lo_p[:], xv3[:, :, 0])
                    nc.scalar.copy(hi_p[:], xv3[:, :, 1])
                    lo_t = planes.tile([P, SIZE], U16, tag="lot")
                    hi_t = planes.tile([P, SIZE], U16, tag="hit")
                    for c in range(N_CHUNK):
                        nc.sync.dma_start_transpose(
                            lo_t[:, c * P : (c + 1) * P],
                            lo_p[:, c * P : (c + 1) * P],
                        )
                        nc.scalar.dma_start_transpose(
                            hi_t[:, c * P : (c + 1) * P],
                            hi_p[:, c * P : (c + 1) * P],
                        )
                    xTm = xtpool.tile([P, SIZE], F32, tag="xtm")
                    m3 = xTm[:].bitcast(U16).rearrange("p (k two) -> p k two", two=2)
                    nc.vector.tensor_copy(m3[:, :, 0], lo_t[:])
                    nc.scalar.copy(m3[:, :, 1], hi_t[:])
                    xT = xtpool.tile([P, SIZE], mdt, tag="xt")
                    nc.scalar.copy(xT[:], xTm[:])  # rounding producer for f32r
                else:
                    # Transpose the 8 [128,128] blocks on the PE; copy to SBUF.
                    xT = xtpool.tile([P, SIZE], mdt, tag="xt")
                    for half in range(2):
                        tp = pst.tile([P, HALF], xdt, tag=f"tp{half}")
                        for q in range(4):
                            c = half * 4 + q
                            nc.tensor.transpose(
                                tp[:, q * P : (q + 1) * P],
                                xv[:, c * P : (c + 1) * P],
                                ident[:],
                            )
                        nc.scalar.copy(xT[:, half * HALF : (half + 1) * HALF], tp[:])

                # out_tile[r, i] = sum_c xT_c^T @ MT_c ; two PSUM banks.
                for h in range(2):
                    po = pso.tile([P, HALF], F32, tag=f"po{h}")
                    for c in range(N_CHUNK):
                        nc.tensor.matmul(
                            po[:],
                            xT[:, c * P : (c + 1) * P],
                            mts[c][:, h * HALF : h * HALF + HALF],
                            start=(c == 0),
                            stop=(c == N_CHUNK - 1),
                        )
                    nc.vector.tensor_copy(
                        osb[:, s * SIZE + h * HALF : s * SIZE + (h + 1) * HALF],
                        po[:],
                    )

            nc.sync.dma_start(
                out[r0 : r0 + P * PAIR, :].rearrange("(s p) n -> p s n", p=P),
                osb[:].rearrange("p (s n) -> p s n", n=SIZE),
            )

    nc.compile()
    return nc


def _get_nc():
    key = MATMUL_MODE
    if key not in _NC_CACHE:
        _NC_CACHE[key] = _build_bass(key)
    return _NC_CACHE[key]


def kernel(x, logits):
    x = np.ascontiguousarray(np.asarray(x), dtype=np.float32)
    logits = np.asarray(logits)
    assert x.shape == (BATCH, SIZE)

    mt = _build_mt(logits)
    nc = _get_nc()

    ident = np.eye(P, dtype=np.float32)
    in_maps = [
        {
            "x": x[i * ROWS_PER_CORE : (i + 1) * ROWS_PER_CORE],
            "mt": mt,
            "ident": ident,
        }
        for i in range(N_CORES)
    ]
    kwargs = dict(TRACE_KWARGS)
    if TRACE:
        kwargs.setdefault("trace", True)
        kwargs.setdefault("trace_cores", [0])
    res = run_bass_kernel_spmd(nc, in_maps, core_ids=list(range(N_CORES)), **kwargs)
    global LAST_RESULTS
    LAST_RESULTS = res
    return np.concatenate([res.results[i]["out"] for i in range(N_CORES)], axis=0)



# revision 6
# speedup vs baseline: 1.3729x; 1.3729x over previous
"""TRN2 Bass kernel for nn_BlockPermProduct.

The reference applies 9 probabilistic block-permutation mixing steps to each
row of x [65536, 1024]. Every step is linear in x, so the whole transform is
``out = x @ M^T`` for a 1024x1024 matrix M depending only on the (9, 3)
logits; M is built on the host in float64 by pushing the identity through the
reference transform.

Two structural tricks take the PE well below the dense-matmul cost:

1. Exact block sparsity. Under the feature grouping g = b0 + 2*b1 + 4*b9
   (bits of the feature index), M has exact zero blocks: outputs in the first
   half (b9=0) never depend on inputs with (b9=1 & b0=1), and outputs in the
   second half never depend on inputs with (b9=0 & b0=0). Each 128-feature
   output block therefore contracts only 6 of the 8 input groups: 48 block
   matmuls instead of 64.

2. Host-side transposes. x is transposed on the host (per core) to
   x_t [1024, 8192], so feature-major tiles DMA straight into SBUF and the
   PE does **only matmuls** — no on-chip transposes at all. The output is
   produced transposed (out_t [1024, 8192]) and un-transposed on the host.

Everything runs in bf16 (rel err ~4e-3 vs the 2e-2 gate): bf16 matmuls are
1 cycle/row like f32r, fp32 accumulation in PSUM, and bf16 I/O halves both
DMA directions. Per 512-row block: one 1 MiB load, 8 output blocks x 6
accumulating 512-wide matmuls, two DVE PSUM->SBUF copies, one 1 MiB store.
PE model: 16 blocks x 48 x 512 cycles = 164 us/core; DMA ~112 us overlapped.

Sharding: pure data parallel over the batch dim across 8 cores (SPMD, no
communication); M is replicated.
"""

import numpy as np
from contextlib import ExitStack

import ml_dtypes

import concourse.bass as bass
import concourse.bacc as bacc
import concourse.mybir as mybir
import concourse.tile as tile
from concourse.bass_utils import run_bass_kernel_spmd

BATCH = 65536
SIZE = 1024
N_CORES = 8
ROWS_PER_CORE = BATCH // N_CORES  # 8192
P = 128
RW = 512  # rows (moving dim) per block
N_STEPS = ROWS_PER_CORE // RW  # 16

F32 = mybir.dt.float32
BF16 = mybir.dt.bfloat16
NP_BF16 = ml_dtypes.bfloat16

# Input-group lists per b9-half of the output (verified against M at runtime;
# dense fallback if the zero pattern does not hold).
KO_HALF0 = [0, 1, 2, 3, 4, 6]  # skip in-groups 5,7 (b9=1 & b0=1)
KO_HALF1 = [1, 3, 4, 5, 6, 7]  # skip in-groups 0,2 (b9=0 & b0=0)

TRACE = False
TRACE_KWARGS = {}
LAST_RESULTS = None

_NC_CACHE = {}


def _transform64(y, logits):
    """Float64 port of the reference transform, applied to rows of y."""
    m = 10
    sizes = [SIZE >> i for i in range(m - 1)][::-1]  # [4, 8, ..., 1024]
    out = y
    for i in range(m - 2, -1, -1):
        n = sizes[i]
        p = 1.0 / (1.0 + np.exp(-logits[i].astype(np.float64)))
        z = out.reshape(-1, n)
        sep = z.reshape(-1, n // 2, 2).transpose(0, 2, 1).reshape(-1, n)
        z = (1 - p[0]) * z + p[0] * sep
        h = n // 2
        first = (1 - p[1]) * z[:, :h] + p[1] * z[:, h - 1::-1]
        second = (1 - p[2]) * z[:, h:] + p[2] * z[:, : h - 1 : -1]
        out = np.concatenate([first, second], axis=1).reshape(out.shape)
    return out


def _build_m(logits):
    """M [1024, 1024] float64: out_row = M @ x_row."""
    eye = np.eye(SIZE, dtype=np.float64)
    mt = _transform64(eye, logits)  # row j = M column j
    return mt.T


def _feat(g, f):
    """Global feature index of element f (0..127) of group g (0..7)."""
    return 512 * (g >> 2) + 4 * f + (g & 3)


_GROUP_FEATS = [np.array([_feat(g, f) for f in range(P)]) for g in range(8)]


def _check_sparse(m):
    """True iff the 48-block zero pattern holds for this M."""
    for o in range(8):
        rows = _GROUP_FEATS[o]
        banned = [5, 7] if o < 4 else [0, 2]
        for i in banned:
            cols = _GROUP_FEATS[i]
            if np.abs(m[np.ix_(rows, cols)]).max() > 1e-12:
                return False
    return True


def _build_mtg(m):
    """Grouped M operand [1024, 1024] bf16.

    mtg[i*128 + f, o*128 + c] = M[_feat(o, c), _feat(i, f)]: row blocks are
    input groups (the matmul contraction dim), column blocks are output
    groups (the matmul stationary free dim).
    """
    mtg = np.zeros((SIZE, SIZE), dtype=np.float64)
    for i in range(8):
        cols = _GROUP_FEATS[i]
        for o in range(8):
            rows = _GROUP_FEATS[o]
            mtg[i * P : (i + 1) * P, o * P : (o + 1) * P] = m[
                np.ix_(rows, cols)
            ].T
    return np.ascontiguousarray(mtg.astype(NP_BF16))


def _build_bass(sparse):
    ko_half = [KO_HALF0, KO_HALF1] if sparse else [list(range(8))] * 2
    nc = bacc.Bacc("TRN2", target_bir_lowering=False, debug=False)
    # x_t / out_t are the per-core transposes: [feature, row].
    xt = nc.dram_tensor("xt", [SIZE, ROWS_PER_CORE], BF16, kind="ExternalInput").ap()
    mtg = nc.dram_tensor("mtg", [SIZE, SIZE], BF16, kind="ExternalInput").ap()
    out = nc.dram_tensor(
        "out_t", [SIZE, ROWS_PER_CORE], BF16, kind="ExternalOutput"
    ).ap()

    with tile.TileContext(nc) as tc, ExitStack() as ctx:
        const = ctx.enter_context(tc.tile_pool(name="const", bufs=1))
        xpool = ctx.enter_context(tc.tile_pool(name="xin", bufs=4))

        def load_x(r0):
            # Four 256 KiB DMAs per 512-row block (group pairs): spreads the
            # transfers across queues and keeps APs at 3 dims. Group g = 4h+q
            # holds features 512h + 4f + q; xin partition=f, free slice
            # g*RW..(g+1)*RW = that group's rows of x_t.
            t = xpool.tile([P, 8 * RW], BF16, tag="xin")
            for gs in range(4):
                h, q0 = divmod(2 * gs, 4)
                src = xt[512 * h : 512 * (h + 1), r0 : r0 + RW].rearrange(
                    "(f q) r -> f q r", q=4
                )
                nc.sync.dma_start(
                    t[:, 2 * gs * RW : 2 * (gs + 1) * RW].rearrange(
                        "p (q r) -> p q r", q=2
                    ),
                    src[:, q0 : q0 + 2, :],
                )
            return t

        # First x load BEFORE the M tiles so the PE isn't queued behind them.
        xin0 = load_x(0)

        mts = []
        for i in range(8):
            t = const.tile([P, SIZE], BF16, tag=f"mt{i}")
            nc.sync.dma_start(t[:], mtg[i * P : (i + 1) * P, :])
            mts.append(t)

        opool = ctx.enter_context(tc.tile_pool(name="osb", bufs=3))
        # Two [128, 4*RW] fp32 tiles per step = exactly the 8 PSUM banks, so a
        # single buf; the DVE copy of half h overlaps the other half's matmuls.
        pso = ctx.enter_context(tc.tile_pool(name="pso", bufs=1, space="PSUM"))

        for step in range(N_STEPS):
            r0 = step * RW
            xin = xin0 if step == 0 else load_x(r0)
            osb = opool.tile([P, 8 * RW], BF16, tag="osb")

            for h in range(2):
                ko = ko_half[h]
                po = pso.tile([P, 4 * RW], F32, tag=f"po{h}")
                for q in range(4):
                    o = 4 * h + q
                    for idx, i in enumerate(ko):
                        nc.tensor.matmul(
                            po[:, q * RW : (q + 1) * RW],
                            mts[i][:, o * P : (o + 1) * P],
                            xin[:, i * RW : (i + 1) * RW],
                            start=(idx == 0),
                            stop=(idx == len(ko) - 1),
                        )
                nc.vector.tensor_copy(osb[:, h * 4 * RW : (h + 1) * 4 * RW], po[:])

            for gs in range(4):
                h, q0 = divmod(2 * gs, 4)
                dst = out[512 * h : 512 * (h + 1), r0 : r0 + RW].rearrange(
                    "(c q) r -> c q r", q=4
                )
                nc.sync.dma_start(
                    dst[:, q0 : q0 + 2, :],
                    osb[:, 2 * gs * RW : 2 * (gs + 1) * RW].rearrange(
                        "p (q r) -> p q r", q=2
                    ),
                )

    nc.compile()
    return nc


def _get_nc(sparse):
    key = bool(sparse)
    if key not in _NC_CACHE:
        _NC_CACHE[key] = _build_bass(key)
    return _NC_CACHE[key]


def kernel(x, logits):
    x = np.asarray(x)
    logits = np.asarray(logits)
    assert x.shape == (BATCH, SIZE)

    m = _build_m(logits)
    sparse = _check_sparse(m)
    mtg = _build_mtg(m)
    nc = _get_nc(sparse)

    xb = x.astype(NP_BF16)
    in_maps = [
        {
            "xt": np.ascontiguousarray(
                xb[i * ROWS_PER_CORE : (i + 1) * ROWS_PER_CORE].T
            ),
            "mtg": mtg,
        }
        for i in range(N_CORES)
    ]
    kwargs = dict(TRACE_KWARGS)
    if TRACE:
        kwargs.setdefault("trace", True)
        kwargs.setdefault("trace_cores", [0])
    res = run_bass_kernel_spmd(nc, in_maps, core_ids=list(range(N_CORES)), **kwargs)
    global LAST_RESULTS
    LAST_RESULTS = res
    outs = [np.asarray(res.results[i]["out_t"]).T for i in range(N_CORES)]
    return np.ascontiguousarray(np.concatenate(outs, axis=0)).astype(np.float32)
